# revision 1
# baseline (speedup 1.0000x reference)
"""Trainium2 Bass kernel for nn_GatedShortBlock (gated depthwise-conv block).

Math (per batch b):
  BCx = x @ w1.T ; Bg, Cg, Xg = split(BCx, 3)
  gated = Bg * Xg
  conv  = causal depthwise conv1d(gated, conv_w, K=4)  (left pad 3)
  out   = (Cg * conv) @ w2.T

Sharding: data-parallel over (batch, seq-half) -> 8 shards of 2048 tokens.
Each core computes its shard fully on-device in a channel-major (transposed)
layout; the 3-token causal halo of `gated` at each shard/block start is
computed on the host (tiny) and shipped as an input.

Matmuls run in float32r (full PE rate, ~1e-3 relative accuracy).
"""

import sys

sys.path.insert(0, "/opt/trn_rl_repo")

import numpy as np
from contextlib import ExitStack

import concourse.bass as bass
import concourse.tile as tile
from concourse import bacc, mybir
from concourse.bass_utils import run_bass_kernel_spmd

F32 = mybir.dt.float32
F32R = mybir.dt.float32r
KS = 4  # conv kernel size
KG = 4  # k-subtiles batched per weight DMA


def build_program(D, E, T, TBLK, CH):
    """One-core program; SPMD across cores with different data."""
    ND = D // 128  # contraction tiles (d)
    NC = D // 128  # channel tiles (c) == output tiles (f)
    NE = E // 128  # w1 output tiles: [Bg: 0..NC-1, Cg: NC..2NC-1, Xg: 2NC..3NC-1]
    NBLK = T // TBLK
    NCH = TBLK // CH  # rhs chunks per weight load (weight reuse)
    assert ND % KG == 0 and NE == 3 * NC

    nc = bacc.Bacc(None)
    xT = nc.dram_tensor("xT", [D, T], F32R, kind="ExternalInput")
    w1T = nc.dram_tensor("w1T", [D, E], F32R, kind="ExternalInput")
    w2T = nc.dram_tensor("w2T", [D, D], F32R, kind="ExternalInput")
    cw = nc.dram_tensor("cw", [D, KS], F32, kind="ExternalInput")
    gh = nc.dram_tensor("gh", [D, KS - 1], F32, kind="ExternalInput")
    outT = nc.dram_tensor("outT", [D, T], F32, kind="ExternalOutput")

    def w_batch_src(w, ncols, e, g):
        # [128 part, KG ksub, 128 m] gather of KG stacked [128,128] tiles:
        # element (p, ks, m) = w[(g*KG+ks)*128 + p, e*128 + m]
        off = (g * KG * 128) * ncols + e * 128
        return bass.AP(w, off, [[ncols, 128], [128 * ncols, KG], [1, 128]])

    with tile.TileContext(nc) as tc, ExitStack() as ctx:
        wp = ctx.enter_context(tc.tile_pool(name="w1p", bufs=8))
        w2p = ctx.enter_context(tc.tile_pool(name="w2p", bufs=4))
        xp = ctx.enter_context(tc.tile_pool(name="xp", bufs=1))
        workp = ctx.enter_context(tc.tile_pool(name="workp", bufs=NC + 2))
        scrp = ctx.enter_context(tc.tile_pool(name="scrp", bufs=3))
        stgp = ctx.enter_context(tc.tile_pool(name="stgp", bufs=3))
        smallp = ctx.enter_context(tc.tile_pool(name="smallp", bufs=1))
        psp = ctx.enter_context(tc.tile_pool(name="psp", bufs=8, space="PSUM"))

        # persistent small tiles: conv weights + gated-halo carry
        cwt = []
        ghsb = []
        for c in range(NC):
            t = smallp.tile([128, KS], F32, tag=f"cw{c}", name=f"cw{c}")
            nc.sync.dma_start(t[:], cw[c * 128 : (c + 1) * 128, :])
            cwt.append(t)
            h = smallp.tile([128, KS - 1], F32, tag=f"gh{c}", name=f"gh{c}")
            ghsb.append(h)

        def load_w_tiles(pool, tag, w, ncols, e):
            tiles = []
            for g in range(ND // KG):
                wt = pool.tile([128, KG * 128], F32R, tag=tag, name=f"{tag}_t")
                nc.sync.dma_start(
                    wt[:].rearrange("p (g m) -> p g m", m=128),
                    w_batch_src(w, ncols, e, g),
                )
                tiles.append(wt)
            return tiles

        def mm_accum(pss, wtiles, rhs_tiles, rhs_col0):
            # pss: NCH psum tiles; accumulate over all ND k-tiles
            for g in range(ND // KG):
                for ks in range(KG):
                    k = g * KG + ks
                    w_ap = wtiles[g][:, ks * 128 : (ks + 1) * 128]
                    for u in range(NCH):
                        nc.tensor.matmul(
                            pss[u][:],
                            w_ap,
                            rhs_tiles[k][:, rhs_col0 + u * CH : rhs_col0 + (u + 1) * CH],
                            start=(k == 0),
                            stop=(k == ND - 1),
                        )

        for b in range(NBLK):
            with nc.named_scope(f"blk{b}"):
                xt = []
                for k in range(ND):
                    t = xp.tile([128, TBLK], F32R, tag=f"x{k}", name=f"x{k}_{b}")
                    nc.sync.dma_start(
                        t[:], xT[k * 128 : (k + 1) * 128, b * TBLK : (b + 1) * TBLK]
                    )
                    xt.append(t)

                # ---- phase A: Bg, Xg -> gated ----
                gwork = []
                for c in range(NC):
                    gw = workp.tile([128, TBLK + KS - 1], F32, tag="work", name=f"gw{b}_{c}")
                    gwork.append(gw)
                    # halo fill
                    if b == 0:
                        nc.sync.dma_start(gw[:, 0 : KS - 1], gh[c * 128 : (c + 1) * 128, :])
                    else:
                        nc.vector.tensor_copy(gw[:, 0 : KS - 1], ghsb[c][:])
                    psB = [
                        psp.tile([128, CH], F32, tag="ps", name=f"psB{b}_{c}_{u}")
                        for u in range(NCH)
                    ]
                    mm_accum(psB, load_w_tiles(wp, "w1", w1T, E, c), xt, 0)
                    psX = [
                        psp.tile([128, CH], F32, tag="ps", name=f"psX{b}_{c}_{u}")
                        for u in range(NCH)
                    ]
                    mm_accum(psX, load_w_tiles(wp, "w1", w1T, E, 2 * NC + c), xt, 0)
                    for u in range(NCH):
                        # DVE reads at most one PSUM operand per instruction:
                        # stage Bg into gwork, then multiply Xg in place.
                        dst = gw[:, KS - 1 + u * CH : KS - 1 + (u + 1) * CH]
                        nc.vector.tensor_copy(dst, psB[u][:])
                        nc.vector.tensor_mul(dst, dst, psX[u][:])

                # ---- phase B: Cg, conv -> R ----
                Rt = []
                for c in range(NC):
                    psC = [
                        psp.tile([128, CH], F32, tag="ps", name=f"psC{b}_{c}_{u}")
                        for u in range(NCH)
                    ]
                    mm_accum(psC, load_w_tiles(wp, "w1", w1T, E, NC + c), xt, 0)
                    gw = gwork[c]
                    s = scrp.tile([128, TBLK], F32, tag="scr", name=f"s0_{b}_{c}")
                    nc.vector.tensor_scalar_mul(s[:], gw[:, 0:TBLK], cwt[c][:, 0:1])
                    for j in range(1, KS):
                        s2 = scrp.tile([128, TBLK], F32, tag="scr", name=f"s{j}_{b}_{c}")
                        nc.vector.scalar_tensor_tensor(
                            s2[:],
                            gw[:, j : j + TBLK],
                            cwt[c][:, j : j + 1],
                            s[:],
                            mybir.AluOpType.mult,
                            mybir.AluOpType.add,
                        )
                        s = s2
                    if b < NBLK - 1:
                        nc.vector.tensor_copy(ghsb[c][:], gw[:, TBLK : TBLK + KS - 1])
                    R = workp.tile([128, TBLK], F32R, tag="work", name=f"R{b}_{c}")
                    Rt.append(R)
                    for u in range(NCH):
                        nc.vector.tensor_mul(
                            R[:, u * CH : (u + 1) * CH],
                            s[:, u * CH : (u + 1) * CH],
                            psC[u][:],
                        )

                # ---- mm2: out = R.T @ w2.T (channel-major) ----
                for f in range(NC):
                    ps2 = [
                        psp.tile([128, CH], F32, tag="ps", name=f"ps2{b}_{f}_{u}")
                        for u in range(NCH)
                    ]
                    for g in range(NC // KG):
                        w2t = w2p.tile([128, KG * 128], F32R, tag="w2", name="w2_t")
                        nc.sync.dma_start(
                            w2t[:].rearrange("p (g m) -> p g m", m=128),
                            w_batch_src(w2T, D, f, g),
                        )
                        for cs in range(KG):
                            c = g * KG + cs
                            w_ap = w2t[:, cs * 128 : (cs + 1) * 128]
                            for u in range(NCH):
                                nc.tensor.matmul(
                                    ps2[u][:],
                                    w_ap,
                                    Rt[c][:, u * CH : (u + 1) * CH],
                                    start=(c == 0),
                                    stop=(c == NC - 1),
                                )
                    for u in range(NCH):
                        st = stgp.tile([128, CH], F32, tag="stg", name=f"st{b}_{f}_{u}")
                        nc.vector.tensor_copy(st[:], ps2[u][:])
                        nc.sync.dma_start(
                            outT[
                                f * 128 : (f + 1) * 128,
                                b * TBLK + u * CH : b * TBLK + (u + 1) * CH,
                            ],
                            st[:],
                        )

    nc.finalize()
    return nc


def shard_inputs(x, w1, w2, conv_w, D, T):
    """Full inputs -> per-core in_maps (channel-major device layouts)."""
    B, S, _ = x.shape
    n_shards = (B * S) // T
    w1T = np.ascontiguousarray(w1.T).astype(np.float32)
    w2T = np.ascontiguousarray(w2.T).astype(np.float32)
    cw = np.ascontiguousarray(conv_w[:, 0, :]).astype(np.float32)
    shards_per_batch = S // T
    in_maps = []
    for s in range(n_shards):
        b, h = divmod(s, shards_per_batch)
        xs = x[b, h * T : (h + 1) * T, :]
        xTs = np.ascontiguousarray(xs.T).astype(np.float32)
        if h == 0:
            ghs = np.zeros((D, KS - 1), np.float32)
        else:
            xh = x[b, h * T - (KS - 1) : h * T, :]
            Bg = xh @ w1[0:D].T
            Xg = xh @ w1[2 * D : 3 * D].T
            ghs = np.ascontiguousarray((Bg * Xg).T).astype(np.float32)
        in_maps.append({"xT": xTs, "w1T": w1T, "w2T": w2T, "cw": cw, "gh": ghs})
    return in_maps


_PROGRAM_CACHE = {}


def run(x, w1, w2, conv_w, D, T, TBLK, CH, trace=False):
    B, S, _ = x.shape
    E = 3 * D
    key = (D, E, T, TBLK, CH)
    if key not in _PROGRAM_CACHE:
        _PROGRAM_CACHE[key] = build_program(D, E, T, TBLK, CH)
    nc = _PROGRAM_CACHE[key]
    in_maps = shard_inputs(x, w1, w2, conv_w, D, T)
    n_shards = len(in_maps)
    res = run_bass_kernel_spmd(nc, in_maps, core_ids=list(range(n_shards)), trace=trace)
    shards_per_batch = S // T
    out = np.empty((B, S, D), np.float32)
    for s in range(n_shards):
        b, h = divmod(s, shards_per_batch)
        out[b, h * T : (h + 1) * T, :] = res.results[s]["outT"].T
    return out, res


def kernel(x, w1, w2, conv_w):
    x = np.asarray(x, np.float32)
    w1 = np.asarray(w1, np.float32)
    w2 = np.asarray(w2, np.float32)
    conv_w = np.asarray(conv_w, np.float32)
    out, _ = run(x, w1, w2, conv_w, D=2048, T=2048, TBLK=1024, CH=512)
    return out



# revision 3
# speedup vs baseline: 1.0800x; 1.0800x over previous
"""Trainium2 Bass kernel for nn_GatedShortBlock (gated depthwise-conv block).

Math (per batch b):
  BCx = x @ w1.T ; Bg, Cg, Xg = split(BCx, 3)
  gated = Bg * Xg
  conv  = causal depthwise conv1d(gated, conv_w, K=4)  (left pad 3)
  out   = (Cg * conv) @ w2.T

Sharding: data-parallel over (batch, seq-half) -> 8 shards of 2048 tokens.
Each core computes its shard fully on-device in a channel-major (transposed)
layout; the 3-token causal halo of `gated` at each shard start is computed
on the host (tiny) and shipped as an input.

v2: fp16 operands (full PE rate, ~6e-4 rel err), single pass over w1/w2
(each weight tile is loaded once and reused for all 4 token chunks),
whole 2048-token shard processed as one block.
"""

import sys

sys.path.insert(0, "/opt/trn_rl_repo")

import numpy as np
from contextlib import ExitStack

import concourse.bass as bass
import concourse.tile as tile
from concourse import bacc, mybir
from concourse.bass_utils import run_bass_kernel_spmd

F32 = mybir.dt.float32
F16 = mybir.dt.float16
KS = 4  # conv kernel size
KG = 4  # k-subtiles batched per weight DMA

D = 2048
E = 3 * D
T = 2048  # tokens per core shard
CH = 512  # psum chunk width
ND = D // 128  # contraction tiles
NC = D // 128  # channel tiles
NCH = T // CH  # chunks per shard


def build_program():
    """One-core program; SPMD across cores with different data."""
    nc = bacc.Bacc(None)
    xT = nc.dram_tensor("xT", [D, T], F16, kind="ExternalInput")
    w1T = nc.dram_tensor("w1T", [D, E], F16, kind="ExternalInput")
    w2T = nc.dram_tensor("w2T", [D, D], F16, kind="ExternalInput")
    cw = nc.dram_tensor("cw", [D, KS], F32, kind="ExternalInput")
    gh = nc.dram_tensor("gh", [D, KS - 1], F16, kind="ExternalInput")
    outT = nc.dram_tensor("outT", [D, T], F32, kind="ExternalOutput")

    def w_batch_src(w, ncols, e, g):
        # [128 part, KG ksub, 128 m] gather of KG stacked [128,128] tiles:
        # element (p, ks, m) = w[(g*KG+ks)*128 + p, e*128 + m]
        off = (g * KG * 128) * ncols + e * 128
        return bass.AP(w, off, [[ncols, 128], [128 * ncols, KG], [1, 128]])

    with tile.TileContext(nc) as tc, ExitStack() as ctx:
        wp = ctx.enter_context(tc.tile_pool(name="wp", bufs=24))
        w2p = ctx.enter_context(tc.tile_pool(name="w2p", bufs=8))
        xp = ctx.enter_context(tc.tile_pool(name="xp", bufs=1))
        rp = ctx.enter_context(tc.tile_pool(name="rp", bufs=1))
        gwp = ctx.enter_context(tc.tile_pool(name="gwp", bufs=3))
        scrp = ctx.enter_context(tc.tile_pool(name="scrp", bufs=3))
        tmpp = ctx.enter_context(tc.tile_pool(name="tmpp", bufs=4))
        stgp = ctx.enter_context(tc.tile_pool(name="stgp", bufs=4))
        smallp = ctx.enter_context(tc.tile_pool(name="smallp", bufs=1))
        psp = ctx.enter_context(tc.tile_pool(name="psp", bufs=8, space="PSUM"))

        # persistent small tiles: conv weights
        cwt = []
        for c in range(NC):
            t = smallp.tile([128, KS], F32, tag=f"cw{c}", name=f"cw{c}")
            nc.sync.dma_start(t[:], cw[c * 128 : (c + 1) * 128, :])
            cwt.append(t)

        # x resident in SBUF (fp16, 64KB/partition)
        xt = []
        for k in range(ND):
            t = xp.tile([128, T], F16, tag=f"x{k}", name=f"x{k}")
            nc.sync.dma_start(t[:], xT[k * 128 : (k + 1) * 128, :])
            xt.append(t)

        def load_w_tiles(pool, tag, w, ncols, e):
            tiles = []
            for g in range(ND // KG):
                wt = pool.tile([128, KG * 128], F16, tag=tag, name=f"{tag}_t")
                nc.sync.dma_start(
                    wt[:].rearrange("p (g m) -> p g m", m=128),
                    w_batch_src(w, ncols, e, g),
                )
                tiles.append(wt)
            return tiles

        def mm_accum(ps, wtiles, col0):
            for g in range(ND // KG):
                for ks in range(KG):
                    k = g * KG + ks
                    nc.tensor.matmul(
                        ps[:],
                        wtiles[g][:, ks * 128 : (ks + 1) * 128],
                        xt[k][:, col0 : col0 + CH],
                        start=(k == 0),
                        stop=(k == ND - 1),
                    )

        Rt = []
        for c in range(NC):
            wB = load_w_tiles(wp, "w1", w1T, E, c)
            wX = load_w_tiles(wp, "w1", w1T, E, 2 * NC + c)
            wC = load_w_tiles(wp, "w1", w1T, E, NC + c)
            gw = gwp.tile([128, T + KS - 1], F16, tag="gw", name=f"gw{c}")
            nc.sync.dma_start(gw[:, 0 : KS - 1], gh[c * 128 : (c + 1) * 128, :])
            for u in range(NCH):
                psB = psp.tile([128, CH], F32, tag="ps", name=f"psB{c}_{u}")
                mm_accum(psB, wB, u * CH)
                psX = psp.tile([128, CH], F32, tag="ps", name=f"psX{c}_{u}")
                mm_accum(psX, wX, u * CH)
                # DVE reads at most one PSUM operand per instruction:
                # stage Bg into SBUF, then multiply with Xg.
                tmp = tmpp.tile([128, CH], F32, tag="tmp", name=f"tmp{c}_{u}")
                nc.vector.tensor_copy(tmp[:], psB[:])
                nc.vector.tensor_mul(
                    gw[:, KS - 1 + u * CH : KS - 1 + (u + 1) * CH], tmp[:], psX[:]
                )
            # depthwise causal conv over gw -> s (fp16, full width)
            s = scrp.tile([128, T], F16, tag="scr", name=f"s0_{c}")
            nc.vector.tensor_scalar_mul(s[:], gw[:, 0:T], cwt[c][:, 0:1])
            for j in range(1, KS):
                s2 = scrp.tile([128, T], F16, tag="scr", name=f"s{j}_{c}")
                nc.vector.scalar_tensor_tensor(
                    s2[:],
                    gw[:, j : j + T],
                    cwt[c][:, j : j + 1],
                    s[:],
                    mybir.AluOpType.mult,
                    mybir.AluOpType.add,
                )
                s = s2
            # R = Cg * conv
            R = rp.tile([128, T], F16, tag=f"R{c}", name=f"R{c}")
            Rt.append(R)
            for u in range(NCH):
                psC = psp.tile([128, CH], F32, tag="ps", name=f"psC{c}_{u}")
                mm_accum(psC, wC, u * CH)
                nc.vector.tensor_mul(
                    R[:, u * CH : (u + 1) * CH],
                    s[:, u * CH : (u + 1) * CH],
                    psC[:],
                )

        # ---- mm2: out = R.T @ w2.T (channel-major) ----
        for f in range(NC):
            w2t = load_w_tiles(w2p, "w2", w2T, D, f)
            for u in range(NCH):
                ps2 = psp.tile([128, CH], F32, tag="ps", name=f"ps2{f}_{u}")
                for g in range(NC // KG):
                    for cs in range(KG):
                        c = g * KG + cs
                        nc.tensor.matmul(
                            ps2[:],
                            w2t[g][:, cs * 128 : (cs + 1) * 128],
                            Rt[c][:, u * CH : (u + 1) * CH],
                            start=(c == 0),
                            stop=(c == NC - 1),
                        )
                st = stgp.tile([128, CH], F32, tag="stg", name=f"st{f}_{u}")
                nc.vector.tensor_copy(st[:], ps2[:])
                nc.sync.dma_start(
                    outT[f * 128 : (f + 1) * 128, u * CH : (u + 1) * CH],
                    st[:],
                )

    nc.finalize()
    return nc


def shard_inputs(x, w1, w2, conv_w):
    """Full inputs -> per-core in_maps (channel-major fp16 device layouts)."""
    B, S, _ = x.shape
    n_shards = (B * S) // T
    w1T = np.ascontiguousarray(w1.T).astype(np.float16)
    w2T = np.ascontiguousarray(w2.T).astype(np.float16)
    cwh = np.ascontiguousarray(conv_w[:, 0, :]).astype(np.float32)
    shards_per_batch = S // T
    in_maps = []
    for s in range(n_shards):
        b, h = divmod(s, shards_per_batch)
        xs = x[b, h * T : (h + 1) * T, :]
        xTs = np.ascontiguousarray(xs.T).astype(np.float16)
        if h == 0:
            ghs = np.zeros((D, KS - 1), np.float16)
        else:
            xh = x[b, h * T - (KS - 1) : h * T, :]
            Bg = xh @ w1[0:D].T
            Xg = xh @ w1[2 * D : 3 * D].T
            ghs = np.ascontiguousarray((Bg * Xg).T).astype(np.float16)
        in_maps.append({"xT": xTs, "w1T": w1T, "w2T": w2T, "cw": cwh, "gh": ghs})
    return in_maps


_PROGRAM_CACHE = {}


def run(x, w1, w2, conv_w, trace=False, **_ignored):
    B, S, _ = x.shape
    if "prog" not in _PROGRAM_CACHE:
        _PROGRAM_CACHE["prog"] = build_program()
    nc = _PROGRAM_CACHE["prog"]
    in_maps = shard_inputs(x, w1, w2, conv_w)
    n_shards = len(in_maps)
    res = run_bass_kernel_spmd(nc, in_maps, core_ids=list(range(n_shards)), trace=trace)
    shards_per_batch = S // T
    out = np.empty((B, S, D), np.float32)
    for s in range(n_shards):
        b, h = divmod(s, shards_per_batch)
        out[b, h * T : (h + 1) * T, :] = res.results[s]["outT"].T
    return out, res


def kernel(x, w1, w2, conv_w):
    x = np.asarray(x, np.float32)
    w1 = np.asarray(w1, np.float32)
    w2 = np.asarray(w2, np.float32)
    conv_w = np.asarray(conv_w, np.float32)
    out, _ = run(x, w1, w2, conv_w)
    return out
